# revision 44
# baseline (speedup 1.0000x reference)
"""Longformer sliding-chunk attention (B=2, S=4096, E=1024, H=16, W=256) on 8 trn2 cores.

Sharding: tensor-parallel over heads — core c owns heads {2c, 2c+1}. Each core:
  - projects q/k/v for its 128 output features (2 heads x 64) over the full
    [8192, 1024] hidden states, in transposed [d, s] layout, bf16
  - computes chunked attention fully transposed and software-pipelined with the
    projections (per 512-seq group: project, transpose new V blocks, compute
    scoresT = K @ Q^T per 128-key-block over its full 768-query window, exp on
    ACT into a bf16 probs ring, then probsT @ V for completed chunks with an
    appended ones-column yielding softmax denominators for free)
  - ships unnormalized numerator^T [128, 8192] + denominators [2, 8192]
Host adds the boundary-mask pad mass to denominators and normalizes.

All matmuls run in bf16 (fp32 PSUM accumulation). No max-subtraction before
exp: scores are O(1) for this problem.
"""
import numpy as np
import ml_dtypes

import concourse.bass as bass
import concourse.mybir as mybir
import concourse.tile as tile
from concourse import bacc
from concourse.bass_utils import run_bass_kernel_spmd
from concourse.masks import make_identity

F32 = mybir.dt.float32
BF16 = mybir.dt.bfloat16
AFT = mybir.ActivationFunctionType

B, S, E = 2, 4096, 1024
H, W, D = 16, 256, 64
BS = B * S           # 8192
KT = 8               # contraction tiles of 128 over E
NCHUNK = S // W      # 16 chunks per batch
NKB = S // 128       # 32 key blocks of 128 per batch
NG = 8               # 512-seq groups per batch
PR = 10              # probs ring slots per head
VR = 16              # vring slots

_NC_CACHE = None


def _score_window(kb):
    """Chunk range [w0, w1) of queries attending key block kb."""
    w0 = max(0, kb // 2 - 1)
    w1 = min(NCHUNK, kb // 2 + 2)
    return w0, w1


def _scores_ready(kb, g):
    """Can scores for key block kb be emitted after local group g of its batch?"""
    have = (g + 1) * 512
    if (kb + 1) * 128 > have:
        return False
    _, w1 = _score_window(kb)
    return w1 * 256 <= have


def _build():
    nc = bacc.Bacc("TRN2", target_bir_lowering=False, debug=False, num_devices=8)

    # host pre-arranges inputs partition-major so every DMA is contiguous
    # per partition: hsT [p, group, ktile, 512], wAll [p, proj, ktile, 128]
    hsT = nc.dram_tensor("hsT", [128, B * NG * KT * 512], BF16,
                         kind="ExternalInput").ap()
    wAll = nc.dram_tensor("wAll", [128, 3 * KT * 128], BF16,
                          kind="ExternalInput").ap()
    bAll = nc.dram_tensor("bAll", [128, 3], F32, kind="ExternalInput").ap()
    outT = nc.dram_tensor("outT", [130, BS], F32, kind="ExternalOutput").ap()

    with tile.TileContext(nc) as tc:
        with (
            tc.tile_pool(name="singles", bufs=1) as singles,
            tc.tile_pool(name="big", bufs=1) as big,
            tc.tile_pool(name="hst", bufs=3) as hpool,
            tc.tile_pool(name="stage", bufs=4) as stage_pool,
            tc.tile_pool(name="den", bufs=4) as den_pool,
            tc.tile_pool(name="psB", bufs=3, space="PSUM") as psB,   # proj/scores/vt
            tc.tile_pool(name="psC", bufs=2, space="PSUM") as psC,   # pv accumulators
        ):
            hsT_r = hsT.rearrange("p (g kt s) -> p g kt s", g=B * NG, kt=KT)

            # weights/biases first (single packed DMAs on SyncE), first input
            # group split per k-tile on GpSimd so the first matmul can start
            # as early as possible
            w_all = singles.tile([128, 3, KT, 128], BF16, tag="wall")
            nc.sync.dma_start(
                out=w_all, in_=wAll.rearrange("p (w kt m) -> p w kt m", w=3, kt=KT)
            )
            b_all = singles.tile([128, 3], F32, tag="ball")
            nc.sync.dma_start(out=b_all, in_=bAll)
            w_sb = {nm: w_all[:, i] for i, nm in enumerate(("q", "k", "v"))}
            b_sb = {nm: b_all[:, i : i + 1] for i, nm in enumerate(("q", "k", "v"))}

            # trigger the ACT table load early so the first real activation
            # doesn't pay for it
            act_warm = singles.tile([128, 1], F32, tag="actwarm")
            nc.scalar.activation(act_warm, b_all[:, 0:1], AFT.Exp)

            ident = singles.tile([128, 128], BF16)
            make_identity(nc, ident)

            # dummy matmuls during the initial DMA wait: keeps the PE busy so
            # the HAM clock gate is already at full rate when real work lands
            warm_ps = psB.tile([128, 1024], F32, tag="mm", name="warmup_ps")
            for _ in range(40):
                nc.tensor.matmul(
                    warm_ps[:, 0:128], lhsT=ident, rhs=ident, start=True, stop=True
                )

            # first group as 8 separate tiles so each matmul only waits on
            # its own k-tile's DMA; issues split across two queues
            gt0_k = []
            for kt in range(KT):
                t0k = singles.tile([128, 512], BF16, tag=f"hst0_{kt}",
                                   name=f"hst0_{kt}")
                eng = nc.gpsimd if kt % 2 == 0 else nc.sync
                eng.dma_start(out=t0k, in_=hsT_r[:, 0, kt, :])
                gt0_k.append(t0k)

            QT = big.tile([128, BS], BF16, tag="qt")
            KTt = big.tile([128, BS], BF16, tag="kt")
            VT = big.tile([128, BS], BF16, tag="vt")
            vring = big.tile([128, VR, 130], BF16, tag="vring")
            # ones columns (softmax denominator trick) via DVE memset: same
            # engine as the V copies, so ordering is total and writes are
            # byte-exact (a DMA here can land late and clobber neighboring
            # V columns at its write granularity)
            nc.vector.memset(
                vring.rearrange("p s (x o) -> p s x o", x=2)[:, :, :, 64:65], 1.0
            )
            probs = {
                h: big.tile([128, PR, 768], BF16, tag=f"probs{h}", name=f"probs{h}")
                for h in (0, 1)
            }

            def emit_transpose(b, kb):
                base = b * S
                slot = (b * NKB + kb) % VR
                vt = psB.tile([128, 128], BF16, tag="mm", name=f"vt_{b}_{kb}")
                nc.tensor.transpose(
                    vt, VT[:, base + kb * 128 : base + (kb + 1) * 128], ident
                )
                nc.vector.tensor_copy(
                    vring[:, slot, :].rearrange("p (h x) -> p h x", h=2)[:, :, 0:64],
                    vt.rearrange("p (h x) -> p h x", h=2),
                )

            def emit_scores(b, kb, h):
                base = b * S
                w0, w1 = _score_window(kb)
                q0 = base + w0 * 256
                width = (w1 - w0) * 256
                d_sl = slice(h * 64, (h + 1) * 64)
                k_sl = slice(base + kb * 128, base + (kb + 1) * 128)
                sp = psB.tile([128, 1024], F32, tag="mm")
                nc.tensor.matmul(
                    sp[:, 0:512],
                    lhsT=KTt[d_sl, k_sl],
                    rhs=QT[d_sl, q0 : q0 + 512],
                    start=True, stop=True,
                )
                if width > 512:
                    nc.tensor.matmul(
                        sp[:, 512:768],
                        lhsT=KTt[d_sl, k_sl],
                        rhs=QT[d_sl, q0 + 512 : q0 + 768],
                        start=True, stop=True,
                    )
                slot = (b * NKB + kb) % PR
                nc.scalar.activation(
                    probs[h][:, slot, 0:width], sp[:, 0:width], AFT.Exp
                )

            def emit_chunk(b, c):
                base = b * S
                lo = max(0, 2 * c - 2)
                hi = min(NKB, 2 * c + 4)
                o_sl = slice(base + c * W, base + (c + 1) * W)
                stage = stage_pool.tile([128, 256], F32, tag="stage")
                po = psC.tile([65, 512], F32, tag="pv")
                for h in (0, 1):
                    for i, kb in enumerate(range(lo, hi)):
                        w0, _ = _score_window(kb)
                        slot = (b * NKB + kb) % PR
                        off = (c - w0) * 256
                        nc.tensor.matmul(
                            po[:, h * 256 : h * 256 + 256],
                            lhsT=vring[
                                :, (b * NKB + kb) % VR, h * 65 : (h + 1) * 65
                            ],
                            rhs=probs[h][:, slot, off : off + 256],
                            start=(i == 0),
                            stop=(i == hi - lo - 1),
                        )
                for h in (0, 1):
                    nc.vector.tensor_copy(
                        stage[h * 64 : (h + 1) * 64, :],
                        po[0:64, h * 256 : h * 256 + 256],
                    )
                    den_h = den_pool.tile(
                        [1, 256], F32, tag=f"den{h}", name=f"den{h}_{b}_{c}"
                    )
                    nc.vector.tensor_copy(den_h, po[64:65, h * 256 : h * 256 + 256])
                    nc.sync.dma_start(out=outT[128 + h : 129 + h, o_sl], in_=den_h)
                nc.sync.dma_start(out=outT[0:128, o_sl], in_=stage)

            # Software pipeline: phase-2 work that becomes ready after a
            # projection sub-group is emitted one sub-group later, so the PE
            # never waits on the just-written Q/K/V of the current group.
            # Each batch: 7 groups of 512 + 2 of 256 (finer tail drain).
            subgroups = []  # (batch, local_seq_offset, width)
            for b in range(B):
                for g in range(NG - 1):
                    subgroups.append((b, g * 512, 512))
                subgroups.append((b, (NG - 1) * 512, 256))
                subgroups.append((b, (NG - 1) * 512 + 256, 256))

            state = [{"kb": 0, "c": 0, "t": 0} for _ in range(B)]

            def phase2_for(b2, seq_done):
                st = state[b2]
                t_kbs = []
                while st["t"] < NKB and (st["t"] + 1) * 128 <= seq_done:
                    t_kbs.append(st["t"])
                    st["t"] += 1
                s_kbs = []
                while st["kb"] < NKB:
                    kb = st["kb"]
                    _, w1 = _score_window(kb)
                    if (kb + 1) * 128 > seq_done or w1 * 256 > seq_done:
                        break
                    s_kbs.append(kb)
                    st["kb"] += 1
                c_done = []
                while st["c"] < NCHUNK and min(NKB, 2 * st["c"] + 4) <= st["kb"]:
                    c_done.append(st["c"])
                    st["c"] += 1
                return t_kbs, s_kbs, c_done

            for gi in range(len(subgroups) + 1):
                if gi < len(subgroups):
                    b, off, width = subgroups[gi]
                    gsl = slice(b * S + off, b * S + off + width)
                    if gi == 0:
                        gt = None
                    else:
                        gt = hpool.tile([128, KT, 512], BF16, tag="hst")
                        g512, rem = divmod(off, 512)
                        nc.gpsimd.dma_start(
                            out=gt[:, :, 0:width],
                            in_=hsT_r[:, b * NG + g512, :, rem : rem + width],
                        )
                    for nm, dest, scale in (
                        ("q", QT, 1.0 / np.sqrt(D)),
                        ("k", KTt, 1.0),
                        ("v", VT, 1.0),
                    ):
                        ps = psB.tile([128, 1024], F32, tag="mm")
                        for kt in range(KT):
                            nc.tensor.matmul(
                                ps[:, 0:width],
                                lhsT=w_sb[nm][:, kt, :],
                                rhs=gt0_k[kt] if gt is None else gt[:, kt, 0:width],
                                start=(kt == 0),
                                stop=(kt == KT - 1),
                            )
                        # bias+scale epilogue on DVE (not ACT) so the exp
                        # stream has the scalar engine to itself
                        nc.vector.tensor_scalar(
                            dest[:, gsl], ps[:, 0:width], scale, b_sb[nm],
                            mybir.AluOpType.mult, mybir.AluOpType.add,
                        )

                if gi == 0:
                    continue
                b2, poff, pwidth = subgroups[gi - 1]
                t_kbs, s_kbs, c_done = phase2_for(b2, poff + pwidth)

                ti = 0
                for kb in s_kbs:
                    if ti < len(t_kbs):
                        emit_transpose(b2, t_kbs[ti])
                        ti += 1
                    for h in (0, 1):
                        emit_scores(b2, kb, h)
                while ti < len(t_kbs):
                    emit_transpose(b2, t_kbs[ti])
                    ti += 1
                for c in c_done:
                    emit_chunk(b2, c)

    nc.compile()
    return nc


def get_nc():
    global _NC_CACHE
    if _NC_CACHE is None:
        _NC_CACHE = _build()
    return _NC_CACHE


def make_in_maps(hidden_states, Wq, bq, Wk, bk, Wv, bv):
    bf16 = ml_dtypes.bfloat16
    # hsT partition-major: [p, group, ktile, 512] flattened to [128, 65536]
    hsT = (
        hidden_states.reshape(BS, E)
        .T.astype(bf16)                       # [E, BS] = [kt*128+p, g*512+x]
        .reshape(KT, 128, B * NG, 512)
        .transpose(1, 2, 0, 3)
        .reshape(128, B * NG * KT * 512)
    )
    in_maps = []
    for c in range(8):
        fsl = slice(c * 128, (c + 1) * 128)
        # wAll partition-major: [p, proj, ktile, 128] flattened to [128, 3072]
        wAll = (
            np.stack(
                [
                    Wm[fsl].T.astype(np.float32).reshape(KT, 128, 128)
                    for Wm in (Wq, Wk, Wv)
                ],
                axis=0,
            )                                  # [w, kt, p, m]
            .transpose(2, 0, 1, 3)
            .reshape(128, 3 * KT * 128)
            .astype(bf16)
        )
        bAll = np.stack(
            [
                bq[fsl].astype(np.float32) / np.sqrt(D),
                bk[fsl].astype(np.float32),
                bv[fsl].astype(np.float32),
            ],
            axis=1,
        )
        in_maps.append(
            {
                "hsT": np.ascontiguousarray(hsT),
                "wAll": np.ascontiguousarray(wAll),
                "bAll": np.ascontiguousarray(bAll),
            }
        )
    return in_maps


def assemble(results):
    """results: list of 8 per-core dicts with 'outT' [130, BS] -> full [B,S,E]."""
    # boundary pad mass: chunk 0 row ii has ii unmasked zero-score pad keys,
    # chunk 15 row ii has 255-ii
    pad = np.zeros(S, np.float32)
    pad[:W] = np.arange(W, dtype=np.float32)
    pad[S - W :] = (W - 1) - np.arange(W, dtype=np.float32)

    out = np.empty((B, S, E), np.float32)
    for c in range(8):
        oT = results[c]["outT"]  # [130, BS]
        num = oT[0:128].T.reshape(B, S, 2, 64)  # b, s, head_local, d
        den = oT[128:130].T.reshape(B, S, 2)  # b, s, head_local
        den = den + pad[None, :, None]
        out[:, :, c * 128 : (c + 1) * 128] = (num / den[..., None]).reshape(B, S, 128)
    return out


def kernel(hidden_states, Wq, bq, Wk, bk, Wv, bv):
    nc = get_nc()
    in_maps = make_in_maps(hidden_states, Wq, bq, Wk, bk, Wv, bv)
    res = run_bass_kernel_spmd(nc, in_maps, list(range(8)))
    return assemble(res.results)


# revision 46
# speedup vs baseline: 1.0188x; 1.0188x over previous
"""Longformer sliding-chunk attention (B=2, S=4096, E=1024, H=16, W=256) on 8 trn2 cores.

Sharding: tensor-parallel over heads — core c owns heads {2c, 2c+1}. Each core:
  - projects q/k/v for its 128 output features (2 heads x 64) over the full
    [8192, 1024] hidden states, in transposed [d, s] layout, bf16
  - computes chunked attention fully transposed and software-pipelined with the
    projections (per 512-seq group: project, transpose new V blocks, compute
    scoresT = K @ Q^T per 128-key-block over its full 768-query window, exp on
    ACT into a bf16 probs ring, then probsT @ V for completed chunks with an
    appended ones-column yielding softmax denominators for free)
  - ships unnormalized numerator^T [128, 8192] + denominators [2, 8192]
Host adds the boundary-mask pad mass to denominators and normalizes.

All matmuls run in bf16 (fp32 PSUM accumulation). No max-subtraction before
exp: scores are O(1) for this problem.
"""
import numpy as np
import ml_dtypes

import concourse.bass as bass
import concourse.mybir as mybir
import concourse.tile as tile
from concourse import bacc
from concourse.bass_utils import run_bass_kernel_spmd
from concourse.masks import make_identity

F32 = mybir.dt.float32
BF16 = mybir.dt.bfloat16
AFT = mybir.ActivationFunctionType

B, S, E = 2, 4096, 1024
H, W, D = 16, 256, 64
BS = B * S           # 8192
KT = 8               # contraction tiles of 128 over E
NCHUNK = S // W      # 16 chunks per batch
NKB = S // 128       # 32 key blocks of 128 per batch
NG = 8               # 512-seq groups per batch
PR = 10              # probs ring slots per head
VR = 16              # vring slots

_NC_CACHE = None


def _score_window(kb):
    """Chunk range [w0, w1) of queries attending key block kb."""
    w0 = max(0, kb // 2 - 1)
    w1 = min(NCHUNK, kb // 2 + 2)
    return w0, w1


def _scores_ready(kb, g):
    """Can scores for key block kb be emitted after local group g of its batch?"""
    have = (g + 1) * 512
    if (kb + 1) * 128 > have:
        return False
    _, w1 = _score_window(kb)
    return w1 * 256 <= have


def _build():
    nc = bacc.Bacc("TRN2", target_bir_lowering=False, debug=False, num_devices=8)

    # host pre-arranges inputs partition-major so every DMA is contiguous
    # per partition: hsT [p, group, ktile, 512], wAll [p, proj, ktile, 128]
    hsT = nc.dram_tensor("hsT", [128, B * NG * KT * 512], BF16,
                         kind="ExternalInput").ap()
    wAll = nc.dram_tensor("wAll", [128, 3 * KT * 128], BF16,
                          kind="ExternalInput").ap()
    bAll = nc.dram_tensor("bAll", [128, 3], F32, kind="ExternalInput").ap()
    outT = nc.dram_tensor("outT", [130, BS], F32, kind="ExternalOutput").ap()

    with tile.TileContext(nc) as tc:
        with (
            tc.tile_pool(name="singles", bufs=1) as singles,
            tc.tile_pool(name="big", bufs=1) as big,
            tc.tile_pool(name="hst", bufs=3) as hpool,
            tc.tile_pool(name="stage", bufs=4) as stage_pool,
            tc.tile_pool(name="den", bufs=4) as den_pool,
            tc.tile_pool(name="psB", bufs=3, space="PSUM") as psB,   # proj/scores/vt
            tc.tile_pool(name="psC", bufs=2, space="PSUM") as psC,   # pv accumulators
        ):
            hsT_r = hsT.rearrange("p (g kt s) -> p g kt s", g=B * NG, kt=KT)

            # weights/biases first (single packed DMAs on SyncE), first input
            # group split per k-tile on GpSimd so the first matmul can start
            # as early as possible
            w_all = singles.tile([128, 3, KT, 128], BF16, tag="wall")
            nc.sync.dma_start(
                out=w_all, in_=wAll.rearrange("p (w kt m) -> p w kt m", w=3, kt=KT)
            )
            b_all = singles.tile([128, 3], F32, tag="ball")
            nc.sync.dma_start(out=b_all, in_=bAll)
            w_sb = {nm: w_all[:, i] for i, nm in enumerate(("q", "k", "v"))}
            b_sb = {nm: b_all[:, i : i + 1] for i, nm in enumerate(("q", "k", "v"))}

            # trigger the ACT table load early so the first real activation
            # doesn't pay for it
            act_warm = singles.tile([128, 1], F32, tag="actwarm")
            nc.scalar.activation(act_warm, b_all[:, 0:1], AFT.Exp)

            ident = singles.tile([128, 128], BF16)
            make_identity(nc, ident)

            # dummy matmuls during the initial DMA wait: keeps the PE busy so
            # the HAM clock gate is already at full rate when real work lands
            warm_ps = psB.tile([128, 1024], F32, tag="mm", name="warmup_ps")
            for _ in range(40):
                nc.tensor.matmul(
                    warm_ps[:, 0:128], lhsT=ident, rhs=ident, start=True, stop=True
                )

            # first group as 8 separate tiles so each matmul only waits on
            # its own k-tile's DMA; issues split across two queues
            gt0_k = []
            for kt in range(KT):
                t0k = singles.tile([128, 512], BF16, tag=f"hst0_{kt}",
                                   name=f"hst0_{kt}")
                eng = nc.gpsimd if kt % 2 == 0 else nc.sync
                eng.dma_start(out=t0k, in_=hsT_r[:, 0, kt, :])
                gt0_k.append(t0k)

            QT = big.tile([128, BS], BF16, tag="qt")
            KTt = big.tile([128, BS], BF16, tag="kt")
            VT = big.tile([128, BS], BF16, tag="vt")
            vring = big.tile([128, VR, 130], BF16, tag="vring")
            # ones columns (softmax denominator trick) via DVE memset: same
            # engine as the V copies, so ordering is total and writes are
            # byte-exact (a DMA here can land late and clobber neighboring
            # V columns at its write granularity)
            nc.vector.memset(
                vring.rearrange("p s (x o) -> p s x o", x=2)[:, :, :, 64:65], 1.0
            )
            probs = {
                h: big.tile([128, PR, 768], BF16, tag=f"probs{h}", name=f"probs{h}")
                for h in (0, 1)
            }

            def emit_transpose(b, kb):
                base = b * S
                slot = (b * NKB + kb) % VR
                vt = psB.tile([128, 128], BF16, tag="mm", name=f"vt_{b}_{kb}")
                nc.tensor.transpose(
                    vt, VT[:, base + kb * 128 : base + (kb + 1) * 128], ident
                )
                nc.vector.tensor_copy(
                    vring[:, slot, :].rearrange("p (h x) -> p h x", h=2)[:, :, 0:64],
                    vt.rearrange("p (h x) -> p h x", h=2),
                )

            def emit_scores(b, kb, h):
                base = b * S
                w0, w1 = _score_window(kb)
                q0 = base + w0 * 256
                width = (w1 - w0) * 256
                d_sl = slice(h * 64, (h + 1) * 64)
                k_sl = slice(base + kb * 128, base + (kb + 1) * 128)
                sp = psB.tile([128, 1024], F32, tag="mm")
                nc.tensor.matmul(
                    sp[:, 0:512],
                    lhsT=KTt[d_sl, k_sl],
                    rhs=QT[d_sl, q0 : q0 + 512],
                    start=True, stop=True,
                )
                if width > 512:
                    nc.tensor.matmul(
                        sp[:, 512:768],
                        lhsT=KTt[d_sl, k_sl],
                        rhs=QT[d_sl, q0 + 512 : q0 + 768],
                        start=True, stop=True,
                    )
                slot = (b * NKB + kb) % PR
                nc.scalar.activation(
                    probs[h][:, slot, 0:width], sp[:, 0:width], AFT.Exp
                )

            def emit_chunk(b, c):
                base = b * S
                lo = max(0, 2 * c - 2)
                hi = min(NKB, 2 * c + 4)
                o_sl = slice(base + c * W, base + (c + 1) * W)
                stage = stage_pool.tile([128, 256], F32, tag="stage")
                po = psC.tile([65, 512], F32, tag="pv")
                for h in (0, 1):
                    for i, kb in enumerate(range(lo, hi)):
                        w0, _ = _score_window(kb)
                        slot = (b * NKB + kb) % PR
                        off = (c - w0) * 256
                        nc.tensor.matmul(
                            po[:, h * 256 : h * 256 + 256],
                            lhsT=vring[
                                :, (b * NKB + kb) % VR, h * 65 : (h + 1) * 65
                            ],
                            rhs=probs[h][:, slot, off : off + 256],
                            start=(i == 0),
                            stop=(i == hi - lo - 1),
                        )
                # stage copies first so the wide output DMA issues before the
                # small denominator copies/DMAs
                for h in (0, 1):
                    nc.vector.tensor_copy(
                        stage[h * 64 : (h + 1) * 64, :],
                        po[0:64, h * 256 : h * 256 + 256],
                    )
                nc.sync.dma_start(out=outT[0:128, o_sl], in_=stage)
                for h in (0, 1):
                    den_h = den_pool.tile(
                        [1, 256], F32, tag=f"den{h}", name=f"den{h}_{b}_{c}"
                    )
                    nc.vector.tensor_copy(den_h, po[64:65, h * 256 : h * 256 + 256])
                    nc.sync.dma_start(out=outT[128 + h : 129 + h, o_sl], in_=den_h)

            # Software pipeline: phase-2 work that becomes ready after a
            # projection sub-group is emitted one sub-group later, so the PE
            # never waits on the just-written Q/K/V of the current group.
            # Each batch: 7 groups of 512 + 2 of 256 (finer tail drain).
            subgroups = []  # (batch, local_seq_offset, width)
            for b in range(B):
                for g in range(NG - 1):
                    subgroups.append((b, g * 512, 512))
                subgroups.append((b, (NG - 1) * 512, 256))
                subgroups.append((b, (NG - 1) * 512 + 256, 256))

            state = [{"kb": 0, "c": 0, "t": 0} for _ in range(B)]

            def phase2_for(b2, seq_done):
                st = state[b2]
                t_kbs = []
                while st["t"] < NKB and (st["t"] + 1) * 128 <= seq_done:
                    t_kbs.append(st["t"])
                    st["t"] += 1
                s_kbs = []
                while st["kb"] < NKB:
                    kb = st["kb"]
                    _, w1 = _score_window(kb)
                    if (kb + 1) * 128 > seq_done or w1 * 256 > seq_done:
                        break
                    s_kbs.append(kb)
                    st["kb"] += 1
                c_done = []
                while st["c"] < NCHUNK and min(NKB, 2 * st["c"] + 4) <= st["kb"]:
                    c_done.append(st["c"])
                    st["c"] += 1
                return t_kbs, s_kbs, c_done

            for gi in range(len(subgroups) + 1):
                if gi < len(subgroups):
                    b, off, width = subgroups[gi]
                    gsl = slice(b * S + off, b * S + off + width)
                    if gi == 0:
                        gt = None
                    else:
                        gt = hpool.tile([128, KT, 512], BF16, tag="hst")
                        g512, rem = divmod(off, 512)
                        nc.gpsimd.dma_start(
                            out=gt[:, :, 0:width],
                            in_=hsT_r[:, b * NG + g512, :, rem : rem + width],
                        )
                    for nm, dest, scale in (
                        ("q", QT, 1.0 / np.sqrt(D)),
                        ("k", KTt, 1.0),
                        ("v", VT, 1.0),
                    ):
                        ps = psB.tile([128, 1024], F32, tag="mm")
                        for kt in range(KT):
                            nc.tensor.matmul(
                                ps[:, 0:width],
                                lhsT=w_sb[nm][:, kt, :],
                                rhs=gt0_k[kt] if gt is None else gt[:, kt, 0:width],
                                start=(kt == 0),
                                stop=(kt == KT - 1),
                            )
                        nc.scalar.activation(
                            dest[:, gsl], ps[:, 0:width], AFT.Identity,
                            bias=b_sb[nm], scale=scale,
                        )

                if gi == 0:
                    continue
                b2, poff, pwidth = subgroups[gi - 1]
                t_kbs, s_kbs, c_done = phase2_for(b2, poff + pwidth)

                ti = 0
                for kb in s_kbs:
                    if ti < len(t_kbs):
                        emit_transpose(b2, t_kbs[ti])
                        ti += 1
                    for h in (0, 1):
                        emit_scores(b2, kb, h)
                while ti < len(t_kbs):
                    emit_transpose(b2, t_kbs[ti])
                    ti += 1
                for c in c_done:
                    emit_chunk(b2, c)

    nc.compile()
    return nc


def get_nc():
    global _NC_CACHE
    if _NC_CACHE is None:
        _NC_CACHE = _build()
    return _NC_CACHE


def make_in_maps(hidden_states, Wq, bq, Wk, bk, Wv, bv):
    bf16 = ml_dtypes.bfloat16
    # hsT partition-major: [p, group, ktile, 512] flattened to [128, 65536]
    hsT = (
        hidden_states.reshape(BS, E)
        .T.astype(bf16)                       # [E, BS] = [kt*128+p, g*512+x]
        .reshape(KT, 128, B * NG, 512)
        .transpose(1, 2, 0, 3)
        .reshape(128, B * NG * KT * 512)
    )
    in_maps = []
    for c in range(8):
        fsl = slice(c * 128, (c + 1) * 128)
        # wAll partition-major: [p, proj, ktile, 128] flattened to [128, 3072]
        wAll = (
            np.stack(
                [
                    Wm[fsl].T.astype(np.float32).reshape(KT, 128, 128)
                    for Wm in (Wq, Wk, Wv)
                ],
                axis=0,
            )                                  # [w, kt, p, m]
            .transpose(2, 0, 1, 3)
            .reshape(128, 3 * KT * 128)
            .astype(bf16)
        )
        bAll = np.stack(
            [
                bq[fsl].astype(np.float32) / np.sqrt(D),
                bk[fsl].astype(np.float32),
                bv[fsl].astype(np.float32),
            ],
            axis=1,
        )
        in_maps.append(
            {
                "hsT": np.ascontiguousarray(hsT),
                "wAll": np.ascontiguousarray(wAll),
                "bAll": np.ascontiguousarray(bAll),
            }
        )
    return in_maps


def assemble(results):
    """results: list of 8 per-core dicts with 'outT' [130, BS] -> full [B,S,E]."""
    # boundary pad mass: chunk 0 row ii has ii unmasked zero-score pad keys,
    # chunk 15 row ii has 255-ii
    pad = np.zeros(S, np.float32)
    pad[:W] = np.arange(W, dtype=np.float32)
    pad[S - W :] = (W - 1) - np.arange(W, dtype=np.float32)

    out = np.empty((B, S, E), np.float32)
    for c in range(8):
        oT = results[c]["outT"]  # [130, BS]
        num = oT[0:128].T.reshape(B, S, 2, 64)  # b, s, head_local, d
        den = oT[128:130].T.reshape(B, S, 2)  # b, s, head_local
        den = den + pad[None, :, None]
        out[:, :, c * 128 : (c + 1) * 128] = (num / den[..., None]).reshape(B, S, 128)
    return out


def kernel(hidden_states, Wq, bq, Wk, bk, Wv, bv):
    nc = get_nc()
    in_maps = make_in_maps(hidden_states, Wq, bq, Wk, bk, Wv, bv)
    res = run_bass_kernel_spmd(nc, in_maps, list(range(8)))
    return assemble(res.results)
